# revision 1
# baseline (speedup 1.0000x reference)
"""Trainium2 Bass kernel for nn_Contrast_loss (B=8192, D=256, 100 classes).

Math: with mask = -same + 0.5*(1-same) + I and same_ii = 1,
    loss = sum((feat @ feat.T) * mask)
         = 0.5*||s||^2 - 1.5*sum_c ||g_c||^2 + sum_i ||f_i||^2
where s = sum_i f_i and g_c = sum_{i: label_i = c} f_i.

Every term decomposes over feature columns, so we shard feat column-wise
across the 8 cores (32 columns each). Each core computes a complete partial
loss over its column slice on device; the host unshards by summing the 8
partial scalars. No cross-core collective is needed.

Per core:
  - g (and s, via an extra all-ones one-hot column) come from a one-hot
    matmul on the tensor engine. feat is split into hi/lo bf16 halves so the
    bf16 matmul products are exact (hi+lo reconstructs fp32 to ~2^-18).
  - sum_i ||f_i||^2 comes from a Square activation with accumulation.
  - the final partition reduction is a [128,1] x ones matmul.
"""

import numpy as np

import concourse.bacc as bacc
import concourse.bass as bass
import concourse.mybir as mybir
import concourse.tile as tile
from concourse import bass_utils

B = 8192
D = 256
N_CORES = 8
DPC = D // N_CORES          # 32 columns per core
P = 128                     # partitions
CHUNKS = B // P             # 64 row chunks of 128
N_GROUPS = 4                # DMA / pipeline groups
CPG = CHUNKS // N_GROUPS    # 16 chunks per group
NCLS = 100                  # label values 0..99
EQ_COLS = 120               # is_equal covers class cols [0, 120); 120 = 4*30
LAMDA = 0.5

FP32 = mybir.dt.float32
BF16 = mybir.dt.bfloat16

_CACHED_NC = None


def _build_nc():
    nc = bacc.Bacc("TRN2", target_bir_lowering=False, debug=False,
                   num_devices=N_CORES)

    feat_d = nc.dram_tensor("feat", [B, DPC], FP32, kind="ExternalInput")
    lab_d = nc.dram_tensor("lab", [P, CHUNKS], FP32, kind="ExternalInput")
    out_d = nc.dram_tensor("out", [1, 1], FP32, kind="ExternalOutput")

    with tile.TileContext(nc) as tc:
        with (
            tc.tile_pool(name="big", bufs=1) as big,
            tc.tile_pool(name="small", bufs=1) as small,
            tc.tile_pool(name="psum", bufs=1, space="PSUM") as psum,
        ):
            # Row r = p*CHUNKS + k lives at (partition p, chunk k).
            feat_t = big.tile([P, CHUNKS, DPC], FP32)     # fp32 feat slice
            hl_t = big.tile([P, CHUNKS, 2 * DPC], BF16)   # [hi | lo] per chunk
            oh_all = big.tile([P, CHUNKS, NCLS + 1], BF16)  # one-hot + ones col
            lab_t = small.tile([P, CHUNKS], FP32)
            iota_t = small.tile([P, P], BF16)             # 0..127 along free
            sq_scratch = big.tile([P, CHUNKS, DPC], BF16)
            sdiag_acc = small.tile([P, N_GROUPS], FP32)

            lab_b16 = small.tile([P, CHUNKS], BF16)
            nc.sync.dma_start(lab_t[:], lab_d.rearrange("p k -> p k"))
            nc.scalar.copy(lab_b16[:], lab_t[:])
            nc.gpsimd.iota(iota_t[:], pattern=[[1, P]], base=0,
                           channel_multiplier=0,
                           allow_small_or_imprecise_dtypes=True)

            # Ones column (computes s in the same matmul) set up front so the
            # is_equal writes (cols 0:100) never overlap it.
            nc.vector.memset(oh_all[:, :, NCLS:NCLS + 1], 1.0)
            # One-hot build: broadcast-compare slices of 8 chunks each on the
            # vector engine, emitted inside the group loop right before their
            # consuming matmuls so the PE tracks DVE production.
            SL = 8

            def emit_oh_slice(s):
                ksl = slice(s * SL, (s + 1) * SL)
                iota_b = iota_t[:, 0:NCLS].unsqueeze(1).broadcast_to(
                    [P, SL, NCLS])
                lab_b = lab_b16[:, ksl].unsqueeze(2).broadcast_to(
                    [P, SL, NCLS])
                nc.vector.tensor_tensor(oh_all[:, ksl, 0:NCLS], iota_b, lab_b,
                                        mybir.AluOpType.is_equal)

            psum_g = psum.tile([NCLS + 1, 2 * DPC], FP32)

            feat_src = feat_d.rearrange("(p k) d -> p k d", p=P)
            for g in range(N_GROUPS):
                ksl = slice(g * CPG, (g + 1) * CPG)
                nc.sync.dma_start(feat_t[:, ksl, :], feat_src[:, ksl, :])
                # hi = bf16(feat); lo = bf16(feat - hi) (lo on gpsimd to keep
                # the vector engine free for the one-hot compares)
                nc.scalar.copy(hl_t[:, ksl, 0:DPC], feat_t[:, ksl, :])
                nc.gpsimd.tensor_sub(hl_t[:, ksl, DPC:2 * DPC],
                                     feat_t[:, ksl, :], hl_t[:, ksl, 0:DPC])
                # sum of squares of this group into sdiag_acc[:, g]
                nc.scalar.activation(sq_scratch[:, ksl, :], feat_t[:, ksl, :],
                                     mybir.ActivationFunctionType.Square,
                                     accum_out=sdiag_acc[:, g:g + 1])
                emit_oh_slice(2 * g)
                emit_oh_slice(2 * g + 1)
                for k in range(g * CPG, (g + 1) * CPG):
                    nc.tensor.matmul(psum_g[:], oh_all[:, k, :], hl_t[:, k, :],
                                     start=(k == 0), stop=(k == CHUNKS - 1))

            # g_sb rows: 0..99 = [g_hi | g_lo] per class, 100 = [s_hi | s_lo]
            NR = NCLS + 1
            g_sb = small.tile([NR, 2 * DPC], FP32)
            nc.scalar.copy(g_sb[:], psum_g[:])
            gt = small.tile([NR, DPC], FP32)
            nc.vector.tensor_add(gt[:], g_sb[:, 0:DPC], g_sb[:, DPC:2 * DPC])
            # q[c] = sum_d g[c,d]^2 ; q[100] = sum_d s_d^2
            # (tensor_tensor_reduce crashes this runtime; use mul + reduce)
            qsc = small.tile([NR, DPC], FP32)
            qq = small.tile([P, 1], FP32)
            nc.vector.memset(qq[:], 0.0)
            nc.vector.tensor_mul(qsc[:], gt[:], gt[:])
            q = qq[0:NR, 0:1]
            nc.vector.tensor_reduce(q, qsc[:], mybir.AxisListType.X,
                                    mybir.AluOpType.add)
            q = qq
            # row weights: -1.5 for class rows, +0.5 for the s row (127),
            # 0 otherwise. Built from a per-partition iota (offset writes
            # must start at an aligned partition, so no direct memsets).
            iota_col = small.tile([P, 1], FP32)
            nc.gpsimd.iota(iota_col[:], pattern=[[0, 1]], base=0,
                           channel_multiplier=1,
                           allow_small_or_imprecise_dtypes=True)
            m1 = small.tile([P, 1], FP32)
            m2 = small.tile([P, 1], FP32)
            w = small.tile([P, 1], FP32)
            nc.vector.tensor_scalar(m1[:], iota_col[:], float(NCLS), None,
                                    mybir.AluOpType.is_lt)
            nc.vector.tensor_scalar(m2[:], iota_col[:], float(NCLS), None,
                                    mybir.AluOpType.is_equal)
            nc.vector.tensor_scalar_mul(m2[:], m2[:], LAMDA)
            nc.vector.scalar_tensor_tensor(
                w[:], m1[:], -(1.0 + LAMDA), m2[:],
                mybir.AluOpType.mult, mybir.AluOpType.add)
            # per-chunk-group diag partials -> [P,1]
            sdiag_vec = small.tile([P, 1], FP32)
            nc.vector.tensor_reduce(sdiag_vec[:], sdiag_acc[:],
                                    mybir.AxisListType.X, mybir.AluOpType.add)
            comb = small.tile([P, 1], FP32)
            nc.vector.tensor_mul(comb[:], q[:], w[:])  # q is the padded qq
            nc.vector.tensor_add(comb[:], comb[:], sdiag_vec[:])
            ones_t = small.tile([P, 1], FP32)
            nc.vector.memset(ones_t[:], 1.0)
            psum_out = psum.tile([1, 1], FP32)
            nc.tensor.matmul(psum_out[:], comb[:], ones_t[:],
                             start=True, stop=True)
            res_t = small.tile([1, 1], FP32)
            nc.scalar.copy(res_t[:], psum_out[:])
            nc.sync.dma_start(out_d[:], res_t[:])

    nc.compile()
    return nc


def _get_nc():
    global _CACHED_NC
    if _CACHED_NC is None:
        _CACHED_NC = _build_nc()
    return _CACHED_NC


def make_in_maps(feat, label):
    feat = np.asarray(feat, dtype=np.float32)
    lab = np.asarray(label).astype(np.float32).reshape(P, CHUNKS)
    return [
        {"feat": np.ascontiguousarray(feat[:, m * DPC:(m + 1) * DPC]),
         "lab": lab}
        for m in range(N_CORES)
    ]


def kernel(feat, label, _trace=False):
    nc = _get_nc()
    in_maps = make_in_maps(feat, label)
    res = bass_utils.run_bass_kernel_spmd(
        nc, in_maps, core_ids=list(range(N_CORES)), trace=_trace)
    total = np.float64(0.0)
    for r in res.results:
        total += np.float64(r["out"][0, 0])
    out = np.float32(total)
    if _trace:
        return out, res
    return out



# revision 5
# speedup vs baseline: 1.1079x; 1.1079x over previous
"""Trainium2 Bass kernel for nn_Contrast_loss (B=8192, D=256, 100 classes).

Math: with mask = -same + 0.5*(1-same) + I and same_ii = 1,
    loss = 0.5*||s||^2 - 1.5*sum_c ||g_c||^2 + sum_i ||f_i||^2
where s = sum_i f_i and g_c = sum_{i: label_i = c} f_i.

Every term decomposes over feature columns, so feat is sharded column-wise
across the 8 cores (32 columns each); the host sums the 8 partial scalars.
No cross-core collective is needed.

Per core, everything runs through one fp8 DoubleRow matmul stream:
  - the host re-encodes label as a one-hot fp8 matrix (exact in fp8) with an
    extra all-ones column (computes s in the same matmul), and feat as an
    fp8 hi/lo pair (hi = e4m3(f), lo = e4m3(f - hi); ~8-bit mantissa total).
  - the PE accumulates G = [onehot|1]^T @ [hi|lo] over 64 row chunks, two
    chunks per DoubleRow matmul.
  - the diag term sum ||f_i||^2 comes from a small DVE pipeline
    (hi+lo -> square -> reduce) overlapped with the matmul stream.
  - tail: q_c = ||g_c||^2 on DVE, weighted sum + diag, then a GpSimd
    partition reduce and a [1,1] DMA out.
"""

import numpy as np
import ml_dtypes

import concourse.bacc as bacc
import concourse.bass as bass
import concourse.mybir as mybir
import concourse.tile as tile
from concourse import bass_utils

B = 8192
D = 256
N_CORES = 8
DPC = D // N_CORES          # 32 feature columns per core
P = 128                     # partitions
CHUNKS = B // P             # 64 row chunks of 128
N_GROUPS = 4                # DMA / pipeline groups
CPG = CHUNKS // N_GROUPS    # 16 chunks per group
NCLS = 100                  # label values 0..99
NR = NCLS + 12              # one-hot cols + ones col + pad to mult-of-16 (dual-fp8 LDW)
LAMDA = 0.5

FP32 = mybir.dt.float32
BF16 = mybir.dt.bfloat16
FP8 = mybir.dt.float8e4
E4M3 = ml_dtypes.float8_e4m3

_CACHED_NC = None


def _build_nc():
    nc = bacc.Bacc("TRN2", target_bir_lowering=False, debug=False,
                   num_devices=N_CORES)

    oh_d = nc.dram_tensor("oh", [B, NR], FP8, kind="ExternalInput")
    fhl_d = nc.dram_tensor("fhl", [B, 2 * DPC], FP8, kind="ExternalInput")
    w_d = nc.dram_tensor("wv", [P, 1], FP32, kind="ExternalInput")
    out_d = nc.dram_tensor("out", [1, 1], FP32, kind="ExternalOutput")

    with tile.TileContext(nc) as tc:
        with (
            tc.tile_pool(name="big", bufs=1) as big,
            tc.tile_pool(name="small", bufs=1) as small,
            tc.tile_pool(name="psum", bufs=1, space="PSUM") as psum,
        ):
            # Row r = p*CHUNKS + k lives at (partition p, chunk k).
            oh_t = big.tile([P, CHUNKS, NR], FP8)
            fhl_t = big.tile([P, CHUNKS, 2 * DPC], FP8)
            fb_t = big.tile([P, CHUNKS, DPC], FP32)
            sq_t = big.tile([P, CHUNKS, DPC], BF16)
            dacc = small.tile([P, N_GROUPS], FP32)
            w_t = small.tile([P, 1], FP32)
            qq = small.tile([P, 1], FP32)

            psum_g = psum.tile([NR, 2 * DPC], FP32)

            nc.sync.dma_start(w_t[:], w_d.rearrange("p c -> p c"))
            nc.vector.memset(qq[:], 0.0)

            oh_src = oh_d.rearrange("(p k) c -> p k c", p=P)
            fhl_src = fhl_d.rearrange("(p k) d -> p k d", p=P)
            for g in range(N_GROUPS):
                ksl = slice(g * CPG, (g + 1) * CPG)
                nc.sync.dma_start(oh_t[:, ksl, :], oh_src[:, ksl, :])
                nc.gpsimd.dma_start(fhl_t[:, ksl, :], fhl_src[:, ksl, :])
                # diag pipeline on DVE (overlapped with the PE stream)
                nc.vector.tensor_add(fb_t[:, ksl, :], fhl_t[:, ksl, 0:DPC],
                                     fhl_t[:, ksl, DPC:2 * DPC])
                nc.vector.tensor_mul(sq_t[:, ksl, :], fb_t[:, ksl, :],
                                     fb_t[:, ksl, :])
                nc.vector.tensor_reduce(dacc[:, g:g + 1], sq_t[:, ksl, :],
                                        mybir.AxisListType.XY,
                                        mybir.AluOpType.add)
                for k in range(g * CPG, (g + 1) * CPG, 2):
                    nc.tensor.matmul(psum_g[:], oh_t[:, k:k + 2, :],
                                     fhl_t[:, k:k + 2, :],
                                     start=(k == 0), stop=(k == CHUNKS - 2),
                                     perf_mode=mybir.MatmulPerfMode.DoubleRow)

            # G rows: 0..99 = [g_hi | g_lo] per class, 100 = [s_hi | s_lo]
            g_sb = small.tile([NR, 2 * DPC], FP32)
            nc.vector.tensor_copy(g_sb[:], psum_g[:])
            gt = small.tile([NR, DPC], FP32)
            nc.vector.tensor_add(gt[:], g_sb[:, 0:DPC], g_sb[:, DPC:2 * DPC])
            qsc = small.tile([NR, DPC], FP32)
            nc.vector.tensor_mul(qsc[:], gt[:], gt[:])
            nc.vector.tensor_reduce(qq[0:NR, 0:1], qsc[:],
                                    mybir.AxisListType.X, mybir.AluOpType.add)
            dsum = small.tile([P, 1], FP32)
            nc.vector.tensor_reduce(dsum[:], dacc[:], mybir.AxisListType.X,
                                    mybir.AluOpType.add)
            comb = small.tile([P, 1], FP32)
            nc.vector.tensor_mul(comb[:], qq[:], w_t[:])
            nc.vector.tensor_add(comb[:], comb[:], dsum[:])
            res_t = small.tile([1, 1], FP32)
            nc.gpsimd.tensor_reduce(res_t[:], comb[:], mybir.AxisListType.C,
                                    mybir.AluOpType.add)
            nc.sync.dma_start(out_d[:], res_t[:])

    nc.compile()
    return nc


def _get_nc():
    global _CACHED_NC
    if _CACHED_NC is None:
        _CACHED_NC = _build_nc()
    return _CACHED_NC


def make_in_maps(feat, label):
    feat = np.asarray(feat, dtype=np.float32)
    lab = np.asarray(label).astype(np.int32)
    oh = (lab[:, None] == np.arange(NR, dtype=np.int32)[None, :])
    oh = oh.astype(E4M3)
    oh[:, NCLS] = E4M3(1.0)            # ones column -> s row
    hi = feat.astype(E4M3)
    lo = (feat - hi.astype(np.float32)).astype(E4M3)
    w = np.zeros((P, 1), dtype=np.float32)
    w[0:NCLS, 0] = -(1.0 + LAMDA)
    w[NCLS, 0] = LAMDA
    maps = []
    for m in range(N_CORES):
        csl = slice(m * DPC, (m + 1) * DPC)
        fhl = np.concatenate([hi[:, csl], lo[:, csl]], axis=1)
        maps.append({"oh": oh, "fhl": np.ascontiguousarray(fhl), "wv": w})
    return maps


def kernel(feat, label, _trace=False):
    nc = _get_nc()
    in_maps = make_in_maps(feat, label)
    res = bass_utils.run_bass_kernel_spmd(
        nc, in_maps, core_ids=list(range(N_CORES)), trace=_trace)
    total = np.float64(0.0)
    for r in res.results:
        total += np.float64(r["out"][0, 0])
    out = np.float32(total)
    if _trace:
        return out, res
    return out
